# revision 27
# baseline (speedup 1.0000x reference)
"""Trainium2 Bass kernel for DeepKoopmanNoDec (8-core SPMD, data-parallel over batch).

Computation (per reference):
  z_k        = concat([x_k, MLP(x_k)])                  # [B, 128]
  z_target   = concat([x_next, MLP(x_next)])            # [B, M, 128]
  bu_m       = u[:, m] @ Bmat                           # [B, 128]
  z_{m+1}    = z_m @ A + bu_m   (scan over M=64)        # z_pred = z_1..z_64
  returns (z_pred_seq [B,M,128], x_pred_seq = z_pred[..., :32], z_target [B,M,128])

Device strategy (per core, batch shard of 256 rows):
  - Feature-major ("transposed") activation layout throughout: [feat, row].
    MLP weights feed the PE as stationary lhsT; no on-device transposes.
  - fp32r matmuls (full PE rate at free-dim >= 256, ~1e-4 rel err).
  - Encoder over 16640 columns (256 x_k cols + 16384 target cols) in
    column-tiles of 512; relu+bias fused, alternating ScalarE/VectorE.
  - The M=64 sequential scan is interleaved 2 steps per encoder tile so its
    serial latency chain hides completely behind encoder PE work.
  - Host does the (cheap) layout transposes during shard/unshard.
"""

import numpy as np

# Problem shapes (hardcoded per spec)
B, MSTEPS = 2048, 64
S, C, E, L, H = 32, 8, 96, 128, 512
N_CORES = 8
BS = B // N_CORES            # 256 batch rows per core
RT = BS * MSTEPS             # 16384 target rows per core
NT = 512                     # encoder column-tile width
N_TILES = RT // NT           # 32
HC = H // 128                # 4 hidden-chunk count

_CACHE = {}


def _build_program():
    import concourse.bacc as bacc
    import concourse.tile as tile
    from concourse import mybir

    f32 = mybir.dt.float32
    f32r = mybir.dt.float32r
    bf16 = mybir.dt.bfloat16
    AT = mybir.ActivationFunctionType
    ALU = mybir.AluOpType

    nc = bacc.Bacc("TRN2", target_bir_lowering=False, debug=False,
                   num_devices=N_CORES)

    # ---- DRAM I/O ----
    xt_d = nc.dram_tensor("xt", [S, BS + RT], bf16, kind="ExternalInput").ap()
    xk32_d = nc.dram_tensor("xk32", [S, BS], f32, kind="ExternalInput").ap()
    ut_d = nc.dram_tensor("ut", [C, MSTEPS * BS], f32, kind="ExternalInput").ap()
    # W1 ships pre-packed [128, 128]: w1[32c+i, j] = W1[i, 128c+j], so the
    # four K=32 first-layer matmuls run concurrently in the PE's four 32-row
    # strips (tile_position row packing) against a 4x-replicated input.
    w1_d = nc.dram_tensor("w1", [128, 128], bf16, kind="ExternalInput").ap()
    w2_d = nc.dram_tensor("w2", [128, HC * H], bf16, kind="ExternalInput").ap()
    w3_d = nc.dram_tensor("w3", [128, HC * H], bf16, kind="ExternalInput").ap()
    wo_d = nc.dram_tensor("wo", [128, HC * 128], bf16, kind="ExternalInput").ap()
    b1_d = nc.dram_tensor("b1", [128, HC], f32, kind="ExternalInput").ap()
    b2_d = nc.dram_tensor("b2", [128, HC], f32, kind="ExternalInput").ap()
    b3_d = nc.dram_tensor("b3", [128, HC], f32, kind="ExternalInput").ap()
    bo_d = nc.dram_tensor("bo", [128, 2], f32, kind="ExternalInput").ap()
    a_d = nc.dram_tensor("a", [L, L], f32, kind="ExternalInput").ap()
    bm_d = nc.dram_tensor("bm", [C, L], f32, kind="ExternalInput").ap()

    et_d = nc.dram_tensor("et", [E, RT], f32, kind="ExternalOutput").ap()
    zp_d = nc.dram_tensor("zp", [MSTEPS, L, BS], f32, kind="ExternalOutput").ap()

    with tile.TileContext(nc) as tc:
        with (
            tc.tile_pool(name="wp", bufs=1) as wp,
            tc.tile_pool(name="xp", bufs=4) as xp,
            tc.tile_pool(name="h1p", bufs=9) as h1p,
            tc.tile_pool(name="h2p", bufs=3) as h2p,
            tc.tile_pool(name="h3p", bufs=3) as h3p,
            tc.tile_pool(name="ep", bufs=3) as ep,
            tc.tile_pool(name="zt", bufs=3) as zpl,
            tc.tile_pool(name="hps", bufs=7, space="PSUM") as hps,
            tc.tile_pool(name="sps", bufs=1, space="PSUM") as sps,
        ):
            # ---- load weights (resident) ----
            w1 = wp.tile([128, 128], bf16, tag="w1")
            w2 = wp.tile([128, HC * H], bf16, tag="w2")
            w3 = wp.tile([128, HC * H], bf16, tag="w3")
            wo = wp.tile([128, HC * 128], bf16, tag="wo")
            b1 = wp.tile([128, HC], f32, tag="b1")
            b2 = wp.tile([128, HC], f32, tag="b2")
            b3 = wp.tile([128, HC], f32, tag="b3")
            bo = wp.tile([128, 2], f32, tag="bo")
            amat = wp.tile([L, L], f32r, tag="amat")
            bmat = wp.tile([C, L], f32r, tag="bmat")
            ut = wp.tile([C, MSTEPS * BS], f32r, tag="ut")
            # DMA order matters for startup latency: w1 + first input tile
            # first (they gate the first matmuls), big weights split into
            # 4 chunks so they spread across DMA queues instead of
            # serializing ~1MB on one queue each.
            nc.sync.dma_start(out=w1, in_=w1_d)
            for sb_t, dr in ((b1, b1_d), (b2, b2_d), (b3, b3_d), (bo, bo_d)):
                nc.sync.dma_start(out=sb_t, in_=dr)

            def dma_chunked(sb_t, dr, nchunks):
                w = dr.shape[-1]
                cw = w // nchunks
                for k in range(nchunks):
                    nc.sync.dma_start(out=sb_t[:, k * cw:(k + 1) * cw],
                                      in_=dr[:, k * cw:(k + 1) * cw].bitcast(sb_t.dtype))

            relu_ix = [0]  # round-robin relu between ScalarE and VectorE

            def relu_bias(out_ap, psum_ap, bias_ap):
                if relu_ix[0] % 2 == 0:
                    nc.scalar.activation(out_ap, psum_ap, AT.Relu, bias=bias_ap)
                else:
                    nc.vector.tensor_scalar(
                        out=out_ap, in0=psum_ap, scalar1=bias_ap, scalar2=0.0,
                        op0=ALU.add, op1=ALU.max)
                relu_ix[0] += 1

            def stage1(c0, n):
                """Load input columns [c0, c0+n) and run layer 1 -> h1 chunks.
                Emitted one tile AHEAD of the rest of the encoder so the PE
                has independent matmuls to chew on while layer-1 relus land."""
                xin = xp.tile([128, NT], bf16, tag="xin", name=f"xin_{c0}")
                load_replicated(xin, c0, n)
                h1 = []
                for c in range(HC):
                    ps = hps.tile([128, NT], f32, tag="hps", name=f"ps1_{c0}_{c}")
                    nc.tensor.matmul(ps[:, :n], w1[32 * c:32 * (c + 1), :],
                                     xin[32 * c:32 * (c + 1), :n],
                                     start=True, stop=True,
                                     tile_position=(32 * c, 0))
                    sb = h1p.tile([128, NT], bf16, tag="h1", name=f"h1_{c0}_{c}")
                    relu_bias(sb[:, :n], ps[:, :n], b1[:, c:c + 1])
                    h1.append(sb)
                return h1

            def encoder_rest(h1, n):
                """Layers 2..4 for a tile whose stage1 ran earlier."""
                prev = h1
                for w, b, pool, tag in ((w2, b2, h2p, "h2"), (w3, b3, h3p, "h3")):
                    cur = []
                    for co in range(HC):
                        ps = hps.tile([128, NT], f32, tag="hps", name=f"ps_{tag}_{co}")
                        for ci in range(HC):
                            nc.tensor.matmul(
                                ps[:, :n],
                                w[:, H * ci + 128 * co: H * ci + 128 * (co + 1)],
                                prev[ci][:, :n],
                                start=(ci == 0), stop=(ci == HC - 1))
                        sb = pool.tile([128, NT], bf16, tag=tag, name=f"sb_{tag}_{co}")
                        relu_bias(sb[:, :n], ps[:, :n], b[:, co:co + 1])
                        cur.append(sb)
                    prev = cur
                # output layer -> [128, n] psum; rows E..127 are zero padding
                ps = hps.tile([128, NT], f32, tag="hps", name="eps_t")
                for ci in range(HC):
                    nc.tensor.matmul(ps[:, :n], wo[:, 128 * ci:128 * (ci + 1)],
                                     prev[ci][:, :n],
                                     start=(ci == 0), stop=(ci == HC - 1))
                return ps

            import concourse.bass as bass

            def load_replicated(xin, c0, n):
                """DMA xt_d[:, c0:c0+n] into xin [128, n], replicated 4x on
                the partition axis (one DMA per replica -> parallel queues)."""
                for c in range(HC):
                    nc.sync.dma_start(out=xin[32 * c:32 * (c + 1), :n],
                                      in_=xt_d[:, c0:c0 + n])

            # ---- x_k tile -> z0 ----
            h1_xk = stage1(0, BS)
            dma_chunked(w2, w2_d, 4)
            dma_chunked(ut, ut_d, 4)
            dma_chunked(w3, w3_d, 4)
            dma_chunked(wo, wo_d, 2)
            nc.sync.dma_start(out=amat, in_=a_d.bitcast(f32r))
            nc.sync.dma_start(out=bmat, in_=bm_d.bitcast(f32r))
            h1_cur = stage1(BS, NT)  # tile 0, one ahead
            e_ps = encoder_rest(h1_xk, BS)
            exk = ep.tile([E, NT], f32, tag="exk")
            nc.vector.tensor_scalar_add(exk[:, :BS], e_ps[:E, :BS], bo[:E, 0:1])
            z_prev = zpl.tile([L, BS], f32r, tag="z")
            nc.sync.dma_start(out=z_prev[:S, :], in_=xk32_d.bitcast(f32r))
            # SBUF->SBUF DMA to shift encoder output down 32 partitions
            nc.sync.dma_start(out=z_prev[S:, :], in_=exk[:, :BS].bitcast(f32r))

            # ---- main loop: 32 encoder tiles, 2 scan steps interleaved ----
            # Each psum block serves TWO scan steps: one N=512 Bmat matmul
            # computes bu for steps 2m and 2m+1, then the two serial A
            # matmuls accumulate into the respective halves.
            state = {"m": 0, "z": z_prev, "ps": None}

            def scan_step():
                m = state["m"]
                if m >= MSTEPS:
                    return
                if m % 2 == 0:
                    ps = sps.tile([L, 2 * BS], f32, tag="sps")
                    nc.tensor.matmul(ps, bmat,
                                     ut[:, m * BS:(m + 2) * BS],
                                     start=True, stop=False)
                    state["ps"] = ps
                else:
                    ps = state["ps"]
                half = ps[:, (m % 2) * BS:(m % 2 + 1) * BS]
                nc.tensor.matmul(half, amat, state["z"],
                                 start=False, stop=(m % 2 == 1))
                z_new = zpl.tile([L, BS], f32r, tag="z")
                nc.vector.tensor_copy(z_new, half)
                nc.sync.dma_start(out=zp_d[m].bitcast(f32r), in_=z_new)
                state["z"] = z_new
                state["m"] = m + 1

            for t in range(N_TILES):
                h1_next = (stage1(BS + (t + 1) * NT, NT)
                           if t + 1 < N_TILES else None)
                e_ps = encoder_rest(h1_cur, NT)
                esb = ep.tile([E, NT], f32, tag="esb")
                nc.vector.tensor_scalar_add(esb, e_ps[:E], bo[:E, 0:1])
                nc.sync.dma_start(out=et_d[:, t * NT:(t + 1) * NT], in_=esb)
                if t >= 2:
                    scan_step()
                    scan_step()
                    scan_step()
                h1_cur = h1_next
            while state["m"] < MSTEPS:
                scan_step()

    nc.compile()
    return nc


def _get_program():
    if "nc" not in _CACHE:
        _CACHE["nc"] = _build_program()
    return _CACHE["nc"]


def kernel(x_k, u_seq, x_next_seq, W1, b1, W2, b2, W3, b3, Wo, bo, A, Bmat):
    outs, _ = run_kernel_internal(
        x_k, u_seq, x_next_seq, W1, b1, W2, b2, W3, b3, Wo, bo, A, Bmat)
    return outs


def run_kernel_internal(x_k, u_seq, x_next_seq, W1, b1, W2, b2, W3, b3,
                        Wo, bo, A, Bmat, **spmd_kwargs):
    from concourse import bass_utils

    x_k = np.asarray(x_k, dtype=np.float32)
    u_seq = np.asarray(u_seq, dtype=np.float32)
    x_next_seq = np.asarray(x_next_seq, dtype=np.float32)
    W1 = np.asarray(W1, dtype=np.float32)
    W2 = np.asarray(W2, dtype=np.float32)
    W3 = np.asarray(W3, dtype=np.float32)
    Wo = np.asarray(Wo, dtype=np.float32)
    b1 = np.asarray(b1, dtype=np.float32)
    b2 = np.asarray(b2, dtype=np.float32)
    b3 = np.asarray(b3, dtype=np.float32)
    bo = np.asarray(bo, dtype=np.float32)
    A = np.asarray(A, dtype=np.float32)
    Bmat = np.asarray(Bmat, dtype=np.float32)

    # ---- replicated weight layouts (encoder in bf16) ----
    import ml_dtypes

    bf = ml_dtypes.bfloat16
    w1b = np.ascontiguousarray(
        W1.reshape(S, HC, 128).transpose(1, 0, 2).reshape(128, 128)).astype(bf)
    w2r = np.ascontiguousarray(
        W2.reshape(HC, 128, H).transpose(1, 0, 2).reshape(128, HC * H)).astype(bf)
    w3r = np.ascontiguousarray(
        W3.reshape(HC, 128, H).transpose(1, 0, 2).reshape(128, HC * H)).astype(bf)
    wo_pad = np.zeros((H, 128), dtype=np.float32)
    wo_pad[:, :E] = Wo
    wor = np.ascontiguousarray(
        wo_pad.reshape(HC, 128, 128).transpose(1, 0, 2).reshape(128, HC * 128)
    ).astype(bf)
    b1r = np.ascontiguousarray(b1.reshape(HC, 128).T)
    b2r = np.ascontiguousarray(b2.reshape(HC, 128).T)
    b3r = np.ascontiguousarray(b3.reshape(HC, 128).T)
    bor = np.zeros((128, 2), dtype=np.float32)
    bor[:E, 0] = bo
    bor[S:, 1] = bo

    shared = {"w1": w1b, "w2": w2r, "w3": w3r, "wo": wor,
              "b1": b1r, "b2": b2r, "b3": b3r, "bo": bor,
              "a": A, "bm": Bmat}

    in_maps = []
    for i in range(N_CORES):
        sl = slice(i * BS, (i + 1) * BS)
        xkT = x_k[sl].T                                        # [32, 256]
        xnT = x_next_seq[sl].reshape(RT, S).T                  # [32, 16384]
        xt = np.ascontiguousarray(
            np.concatenate([xkT, xnT], axis=1)).astype(bf)
        xk32 = np.ascontiguousarray(xkT)
        ut = np.ascontiguousarray(
            u_seq[sl].transpose(2, 1, 0).reshape(C, MSTEPS * BS))
        in_maps.append({"xt": xt, "xk32": xk32, "ut": ut, **shared})

    nc = _get_program()
    res = bass_utils.run_bass_kernel_spmd(
        nc, in_maps, core_ids=list(range(N_CORES)), **spmd_kwargs)

    z_pred = np.empty((B, MSTEPS, L), dtype=np.float32)
    z_target = np.empty((B, MSTEPS, L), dtype=np.float32)
    for i in range(N_CORES):
        sl = slice(i * BS, (i + 1) * BS)
        out = res.results[i]
        z_pred[sl] = out["zp"].transpose(2, 0, 1)              # [256, 64, 128]
        z_target[sl, :, :S] = x_next_seq[sl]
        z_target[sl, :, S:] = out["et"].T.reshape(BS, MSTEPS, E)
    x_pred = np.ascontiguousarray(z_pred[..., :S])
    return (z_pred, x_pred, z_target), res


# revision 28
# speedup vs baseline: 1.0662x; 1.0662x over previous
"""Trainium2 Bass kernel for DeepKoopmanNoDec (8-core SPMD, data-parallel over batch).

Computation (per reference):
  z_k        = concat([x_k, MLP(x_k)])                  # [B, 128]
  z_target   = concat([x_next, MLP(x_next)])            # [B, M, 128]
  bu_m       = u[:, m] @ Bmat                           # [B, 128]
  z_{m+1}    = z_m @ A + bu_m   (scan over M=64)        # z_pred = z_1..z_64
  returns (z_pred_seq [B,M,128], x_pred_seq = z_pred[..., :32], z_target [B,M,128])

Device strategy (per core, batch shard of 256 rows):
  - Feature-major ("transposed") activation layout throughout: [feat, row].
    MLP weights feed the PE as stationary lhsT; no on-device transposes.
  - fp32r matmuls (full PE rate at free-dim >= 256, ~1e-4 rel err).
  - Encoder over 16640 columns (256 x_k cols + 16384 target cols) in
    column-tiles of 512; relu+bias fused, alternating ScalarE/VectorE.
  - The M=64 sequential scan is interleaved 2 steps per encoder tile so its
    serial latency chain hides completely behind encoder PE work.
  - Host does the (cheap) layout transposes during shard/unshard.
"""

import numpy as np

# Problem shapes (hardcoded per spec)
B, MSTEPS = 2048, 64
S, C, E, L, H = 32, 8, 96, 128, 512
N_CORES = 8
BS = B // N_CORES            # 256 batch rows per core
RT = BS * MSTEPS             # 16384 target rows per core
NT = 512                     # encoder column-tile width
N_TILES = RT // NT           # 32
HC = H // 128                # 4 hidden-chunk count

_CACHE = {}


def _build_program():
    import concourse.bacc as bacc
    import concourse.tile as tile
    from concourse import mybir

    f32 = mybir.dt.float32
    f32r = mybir.dt.float32r
    bf16 = mybir.dt.bfloat16
    AT = mybir.ActivationFunctionType
    ALU = mybir.AluOpType

    nc = bacc.Bacc("TRN2", target_bir_lowering=False, debug=False,
                   num_devices=N_CORES)

    # ---- DRAM I/O ----
    xt_d = nc.dram_tensor("xt", [S, BS + RT], bf16, kind="ExternalInput").ap()
    xk32_d = nc.dram_tensor("xk32", [S, BS], f32, kind="ExternalInput").ap()
    ut_d = nc.dram_tensor("ut", [C, MSTEPS * BS], f32, kind="ExternalInput").ap()
    # W1 ships pre-packed [128, 128]: w1[32c+i, j] = W1[i, 128c+j], so the
    # four K=32 first-layer matmuls run concurrently in the PE's four 32-row
    # strips (tile_position row packing) against a 4x-replicated input.
    w1_d = nc.dram_tensor("w1", [128, 128], bf16, kind="ExternalInput").ap()
    w2_d = nc.dram_tensor("w2", [128, HC * H], bf16, kind="ExternalInput").ap()
    w3_d = nc.dram_tensor("w3", [128, HC * H], bf16, kind="ExternalInput").ap()
    wo_d = nc.dram_tensor("wo", [128, HC * 128], bf16, kind="ExternalInput").ap()
    b1_d = nc.dram_tensor("b1", [128, HC], f32, kind="ExternalInput").ap()
    b2_d = nc.dram_tensor("b2", [128, HC], f32, kind="ExternalInput").ap()
    b3_d = nc.dram_tensor("b3", [128, HC], f32, kind="ExternalInput").ap()
    bo_d = nc.dram_tensor("bo", [128, 2], f32, kind="ExternalInput").ap()
    a_d = nc.dram_tensor("a", [L, L], f32, kind="ExternalInput").ap()
    bm_d = nc.dram_tensor("bm", [C, L], f32, kind="ExternalInput").ap()

    et_d = nc.dram_tensor("et", [E, RT], f32, kind="ExternalOutput").ap()
    zp_d = nc.dram_tensor("zp", [MSTEPS, L, BS], f32, kind="ExternalOutput").ap()

    with tile.TileContext(nc) as tc:
        with (
            tc.tile_pool(name="wp", bufs=1) as wp,
            tc.tile_pool(name="xp", bufs=4) as xp,
            tc.tile_pool(name="h1p", bufs=9) as h1p,
            tc.tile_pool(name="h2p", bufs=3) as h2p,
            tc.tile_pool(name="h3p", bufs=3) as h3p,
            tc.tile_pool(name="ep", bufs=3) as ep,
            tc.tile_pool(name="zt", bufs=3) as zpl,
            tc.tile_pool(name="hps", bufs=7, space="PSUM") as hps,
            tc.tile_pool(name="sps", bufs=1, space="PSUM") as sps,
        ):
            # ---- load weights (resident) ----
            w1 = wp.tile([128, 128], bf16, tag="w1")
            w2 = wp.tile([128, HC * H], bf16, tag="w2")
            w3 = wp.tile([128, HC * H], bf16, tag="w3")
            wo = wp.tile([128, HC * 128], bf16, tag="wo")
            b1 = wp.tile([128, HC], f32, tag="b1")
            b2 = wp.tile([128, HC], f32, tag="b2")
            b3 = wp.tile([128, HC], f32, tag="b3")
            bo = wp.tile([128, 2], f32, tag="bo")
            amat = wp.tile([L, L], f32r, tag="amat")
            bmat = wp.tile([C, L], f32r, tag="bmat")
            ut = wp.tile([C, MSTEPS * BS], f32r, tag="ut")
            # DMA order matters for startup latency: w1 + first input tile
            # first (they gate the first matmuls), big weights split into
            # 4 chunks so they spread across DMA queues instead of
            # serializing ~1MB on one queue each.
            nc.sync.dma_start(out=w1, in_=w1_d)
            for sb_t, dr in ((b1, b1_d), (b2, b2_d), (b3, b3_d), (bo, bo_d)):
                nc.sync.dma_start(out=sb_t, in_=dr)

            def dma_chunked(sb_t, dr, nchunks):
                w = dr.shape[-1]
                cw = w // nchunks
                for k in range(nchunks):
                    nc.sync.dma_start(out=sb_t[:, k * cw:(k + 1) * cw],
                                      in_=dr[:, k * cw:(k + 1) * cw].bitcast(sb_t.dtype))

            relu_ix = [0]  # round-robin relu between ScalarE and VectorE

            def relu_bias(out_ap, psum_ap, bias_ap):
                if relu_ix[0] % 2 == 0:
                    nc.scalar.activation(out_ap, psum_ap, AT.Relu, bias=bias_ap)
                else:
                    nc.vector.tensor_scalar(
                        out=out_ap, in0=psum_ap, scalar1=bias_ap, scalar2=0.0,
                        op0=ALU.add, op1=ALU.max)
                relu_ix[0] += 1

            def stage1(c0, n):
                """Load input columns [c0, c0+n) and run layer 1 -> h1 chunks.
                Emitted one tile AHEAD of the rest of the encoder so the PE
                has independent matmuls to chew on while layer-1 relus land."""
                xin = xp.tile([128, NT], bf16, tag="xin", name=f"xin_{c0}")
                load_replicated(xin, c0, n)
                h1 = []
                for c in range(HC):
                    ps = hps.tile([128, NT], f32, tag="hps", name=f"ps1_{c0}_{c}")
                    nc.tensor.matmul(ps[:, :n], w1[32 * c:32 * (c + 1), :],
                                     xin[32 * c:32 * (c + 1), :n],
                                     start=True, stop=True,
                                     tile_position=(32 * c, 0))
                    sb = h1p.tile([128, NT], bf16, tag="h1", name=f"h1_{c0}_{c}")
                    relu_bias(sb[:, :n], ps[:, :n], b1[:, c:c + 1])
                    h1.append(sb)
                return h1

            def encoder_rest(h1, n):
                """Layers 2..4 for a tile whose stage1 ran earlier."""
                prev = h1
                for w, b, pool, tag in ((w2, b2, h2p, "h2"), (w3, b3, h3p, "h3")):
                    cur = []
                    for co in range(HC):
                        ps = hps.tile([128, NT], f32, tag="hps", name=f"ps_{tag}_{co}")
                        for ci in range(HC):
                            nc.tensor.matmul(
                                ps[:, :n],
                                w[:, H * ci + 128 * co: H * ci + 128 * (co + 1)],
                                prev[ci][:, :n],
                                start=(ci == 0), stop=(ci == HC - 1))
                        sb = pool.tile([128, NT], bf16, tag=tag, name=f"sb_{tag}_{co}")
                        relu_bias(sb[:, :n], ps[:, :n], b[:, co:co + 1])
                        cur.append(sb)
                    prev = cur
                # output layer -> [128, n] psum; rows E..127 are zero padding
                ps = hps.tile([128, NT], f32, tag="hps", name="eps_t")
                for ci in range(HC):
                    nc.tensor.matmul(ps[:, :n], wo[:, 128 * ci:128 * (ci + 1)],
                                     prev[ci][:, :n],
                                     start=(ci == 0), stop=(ci == HC - 1))
                return ps

            import concourse.bass as bass

            def load_replicated(xin, c0, n):
                """DMA xt_d[:, c0:c0+n] into xin [128, n], replicated 4x on
                the partition axis (one DMA per replica -> parallel queues)."""
                for c in range(HC):
                    nc.sync.dma_start(out=xin[32 * c:32 * (c + 1), :n],
                                      in_=xt_d[:, c0:c0 + n])

            # ---- x_k tile -> z0 ----
            h1_xk = stage1(0, BS)
            dma_chunked(w2, w2_d, 4)
            dma_chunked(ut, ut_d, 4)
            dma_chunked(w3, w3_d, 4)
            dma_chunked(wo, wo_d, 2)
            nc.sync.dma_start(out=amat, in_=a_d.bitcast(f32r))
            nc.sync.dma_start(out=bmat, in_=bm_d.bitcast(f32r))
            h1_cur = stage1(BS, NT)  # tile 0, one ahead
            e_ps = encoder_rest(h1_xk, BS)
            exk = ep.tile([E, NT], f32, tag="exk")
            nc.vector.tensor_scalar_add(exk[:, :BS], e_ps[:E, :BS], bo[:E, 0:1])
            z_prev = zpl.tile([L, BS], f32r, tag="z")
            nc.sync.dma_start(out=z_prev[:S, :], in_=xk32_d.bitcast(f32r))
            # SBUF->SBUF DMA to shift encoder output down 32 partitions
            nc.sync.dma_start(out=z_prev[S:, :], in_=exk[:, :BS].bitcast(f32r))

            # ---- main loop: 32 encoder tiles, 2 scan steps interleaved ----
            # Each psum block serves TWO scan steps: one N=512 Bmat matmul
            # computes bu for steps 2m and 2m+1, then the two serial A
            # matmuls accumulate into the respective halves.
            state = {"m": 0, "z": z_prev, "ps": None}

            def scan_step():
                m = state["m"]
                if m >= MSTEPS:
                    return
                if m % 2 == 0:
                    ps = sps.tile([L, 2 * BS], f32, tag="sps")
                    nc.tensor.matmul(ps, bmat,
                                     ut[:, m * BS:(m + 2) * BS],
                                     start=True, stop=False)
                    state["ps"] = ps
                else:
                    ps = state["ps"]
                half = ps[:, (m % 2) * BS:(m % 2 + 1) * BS]
                nc.tensor.matmul(half, amat, state["z"],
                                 start=False, stop=(m % 2 == 1))
                z_new = zpl.tile([L, BS], f32r, tag="z")
                nc.vector.tensor_copy(z_new, half)
                nc.sync.dma_start(out=zp_d[m].bitcast(f32r), in_=z_new)
                state["z"] = z_new
                state["m"] = m + 1

            for t in range(N_TILES):
                h1_next = (stage1(BS + (t + 1) * NT, NT)
                           if t + 1 < N_TILES else None)
                # scan steps bracket the encoder body so consecutive steps
                # have ~36 matmuls between them (hides the psum->sbuf->PE
                # round-trip of the serial z chain)
                if t >= 1:
                    scan_step()
                e_ps = encoder_rest(h1_cur, NT)
                esb = ep.tile([E, NT], f32, tag="esb")
                nc.vector.tensor_scalar_add(esb, e_ps[:E], bo[:E, 0:1])
                nc.sync.dma_start(out=et_d[:, t * NT:(t + 1) * NT], in_=esb)
                if t >= 1:
                    scan_step()
                h1_cur = h1_next
            while state["m"] < MSTEPS:
                scan_step()

    nc.compile()
    return nc


def _get_program():
    if "nc" not in _CACHE:
        _CACHE["nc"] = _build_program()
    return _CACHE["nc"]


def kernel(x_k, u_seq, x_next_seq, W1, b1, W2, b2, W3, b3, Wo, bo, A, Bmat):
    outs, _ = run_kernel_internal(
        x_k, u_seq, x_next_seq, W1, b1, W2, b2, W3, b3, Wo, bo, A, Bmat)
    return outs


def run_kernel_internal(x_k, u_seq, x_next_seq, W1, b1, W2, b2, W3, b3,
                        Wo, bo, A, Bmat, **spmd_kwargs):
    from concourse import bass_utils

    x_k = np.asarray(x_k, dtype=np.float32)
    u_seq = np.asarray(u_seq, dtype=np.float32)
    x_next_seq = np.asarray(x_next_seq, dtype=np.float32)
    W1 = np.asarray(W1, dtype=np.float32)
    W2 = np.asarray(W2, dtype=np.float32)
    W3 = np.asarray(W3, dtype=np.float32)
    Wo = np.asarray(Wo, dtype=np.float32)
    b1 = np.asarray(b1, dtype=np.float32)
    b2 = np.asarray(b2, dtype=np.float32)
    b3 = np.asarray(b3, dtype=np.float32)
    bo = np.asarray(bo, dtype=np.float32)
    A = np.asarray(A, dtype=np.float32)
    Bmat = np.asarray(Bmat, dtype=np.float32)

    # ---- replicated weight layouts (encoder in bf16) ----
    import ml_dtypes

    bf = ml_dtypes.bfloat16
    w1b = np.ascontiguousarray(
        W1.reshape(S, HC, 128).transpose(1, 0, 2).reshape(128, 128)).astype(bf)
    w2r = np.ascontiguousarray(
        W2.reshape(HC, 128, H).transpose(1, 0, 2).reshape(128, HC * H)).astype(bf)
    w3r = np.ascontiguousarray(
        W3.reshape(HC, 128, H).transpose(1, 0, 2).reshape(128, HC * H)).astype(bf)
    wo_pad = np.zeros((H, 128), dtype=np.float32)
    wo_pad[:, :E] = Wo
    wor = np.ascontiguousarray(
        wo_pad.reshape(HC, 128, 128).transpose(1, 0, 2).reshape(128, HC * 128)
    ).astype(bf)
    b1r = np.ascontiguousarray(b1.reshape(HC, 128).T)
    b2r = np.ascontiguousarray(b2.reshape(HC, 128).T)
    b3r = np.ascontiguousarray(b3.reshape(HC, 128).T)
    bor = np.zeros((128, 2), dtype=np.float32)
    bor[:E, 0] = bo
    bor[S:, 1] = bo

    shared = {"w1": w1b, "w2": w2r, "w3": w3r, "wo": wor,
              "b1": b1r, "b2": b2r, "b3": b3r, "bo": bor,
              "a": A, "bm": Bmat}

    in_maps = []
    for i in range(N_CORES):
        sl = slice(i * BS, (i + 1) * BS)
        xkT = x_k[sl].T                                        # [32, 256]
        xnT = x_next_seq[sl].reshape(RT, S).T                  # [32, 16384]
        xt = np.ascontiguousarray(
            np.concatenate([xkT, xnT], axis=1)).astype(bf)
        xk32 = np.ascontiguousarray(xkT)
        ut = np.ascontiguousarray(
            u_seq[sl].transpose(2, 1, 0).reshape(C, MSTEPS * BS))
        in_maps.append({"xt": xt, "xk32": xk32, "ut": ut, **shared})

    nc = _get_program()
    res = bass_utils.run_bass_kernel_spmd(
        nc, in_maps, core_ids=list(range(N_CORES)), **spmd_kwargs)

    z_pred = np.empty((B, MSTEPS, L), dtype=np.float32)
    z_target = np.empty((B, MSTEPS, L), dtype=np.float32)
    for i in range(N_CORES):
        sl = slice(i * BS, (i + 1) * BS)
        out = res.results[i]
        z_pred[sl] = out["zp"].transpose(2, 0, 1)              # [256, 64, 128]
        z_target[sl, :, :S] = x_next_seq[sl]
        z_target[sl, :, S:] = out["et"].T.reshape(BS, MSTEPS, E)
    x_pred = np.ascontiguousarray(z_pred[..., :S])
    return (z_pred, x_pred, z_target), res


# revision 29
# speedup vs baseline: 1.0682x; 1.0019x over previous
"""Trainium2 Bass kernel for DeepKoopmanNoDec (8-core SPMD, data-parallel over batch).

Computation (per reference):
  z_k        = concat([x_k, MLP(x_k)])                  # [B, 128]
  z_target   = concat([x_next, MLP(x_next)])            # [B, M, 128]
  bu_m       = u[:, m] @ Bmat                           # [B, 128]
  z_{m+1}    = z_m @ A + bu_m   (scan over M=64)        # z_pred = z_1..z_64
  returns (z_pred_seq [B,M,128], x_pred_seq = z_pred[..., :32], z_target [B,M,128])

Device strategy (per core, batch shard of 256 rows):
  - Feature-major ("transposed") activation layout throughout: [feat, row].
    MLP weights feed the PE as stationary lhsT; no on-device transposes.
  - fp32r matmuls (full PE rate at free-dim >= 256, ~1e-4 rel err).
  - Encoder over 16640 columns (256 x_k cols + 16384 target cols) in
    column-tiles of 512; relu+bias fused, alternating ScalarE/VectorE.
  - The M=64 sequential scan is interleaved 2 steps per encoder tile so its
    serial latency chain hides completely behind encoder PE work.
  - Host does the (cheap) layout transposes during shard/unshard.
"""

import numpy as np

# Problem shapes (hardcoded per spec)
B, MSTEPS = 2048, 64
S, C, E, L, H = 32, 8, 96, 128, 512
N_CORES = 8
BS = B // N_CORES            # 256 batch rows per core
RT = BS * MSTEPS             # 16384 target rows per core
NT = 512                     # encoder column-tile width
N_TILES = RT // NT           # 32
HC = H // 128                # 4 hidden-chunk count

_CACHE = {}


def _build_program():
    import concourse.bacc as bacc
    import concourse.tile as tile
    from concourse import mybir

    f32 = mybir.dt.float32
    f32r = mybir.dt.float32r
    bf16 = mybir.dt.bfloat16
    AT = mybir.ActivationFunctionType
    ALU = mybir.AluOpType

    nc = bacc.Bacc("TRN2", target_bir_lowering=False, debug=False,
                   num_devices=N_CORES)

    # ---- DRAM I/O ----
    xt_d = nc.dram_tensor("xt", [S, BS + RT], bf16, kind="ExternalInput").ap()
    xk32_d = nc.dram_tensor("xk32", [S, BS], f32, kind="ExternalInput").ap()
    ut_d = nc.dram_tensor("ut", [C, MSTEPS * BS], f32, kind="ExternalInput").ap()
    # W1 ships pre-packed [128, 128]: w1[32c+i, j] = W1[i, 128c+j], so the
    # four K=32 first-layer matmuls run concurrently in the PE's four 32-row
    # strips (tile_position row packing) against a 4x-replicated input.
    w1_d = nc.dram_tensor("w1", [128, 128], bf16, kind="ExternalInput").ap()
    w2_d = nc.dram_tensor("w2", [128, HC * H], bf16, kind="ExternalInput").ap()
    w3_d = nc.dram_tensor("w3", [128, HC * H], bf16, kind="ExternalInput").ap()
    wo_d = nc.dram_tensor("wo", [128, HC * 128], bf16, kind="ExternalInput").ap()
    b1_d = nc.dram_tensor("b1", [128, HC], f32, kind="ExternalInput").ap()
    b2_d = nc.dram_tensor("b2", [128, HC], f32, kind="ExternalInput").ap()
    b3_d = nc.dram_tensor("b3", [128, HC], f32, kind="ExternalInput").ap()
    bo_d = nc.dram_tensor("bo", [128, 2], f32, kind="ExternalInput").ap()
    a_d = nc.dram_tensor("a", [L, L], f32, kind="ExternalInput").ap()
    bm_d = nc.dram_tensor("bm", [C, L], f32, kind="ExternalInput").ap()

    et_d = nc.dram_tensor("et", [E, RT], f32, kind="ExternalOutput").ap()
    zp_d = nc.dram_tensor("zp", [MSTEPS, L, BS], f32, kind="ExternalOutput").ap()

    with tile.TileContext(nc) as tc:
        with (
            tc.tile_pool(name="wp", bufs=1) as wp,
            tc.tile_pool(name="xp", bufs=4) as xp,
            tc.tile_pool(name="h1p", bufs=9) as h1p,
            tc.tile_pool(name="h2p", bufs=3) as h2p,
            tc.tile_pool(name="h3p", bufs=3) as h3p,
            tc.tile_pool(name="ep", bufs=3) as ep,
            tc.tile_pool(name="zt", bufs=3) as zpl,
            tc.tile_pool(name="hps", bufs=7, space="PSUM") as hps,
            tc.tile_pool(name="sps", bufs=1, space="PSUM") as sps,
        ):
            # ---- load weights (resident) ----
            w1 = wp.tile([128, 128], bf16, tag="w1")
            w2 = wp.tile([128, HC * H], bf16, tag="w2")
            w3 = wp.tile([128, HC * H], bf16, tag="w3")
            wo = wp.tile([128, HC * 128], bf16, tag="wo")
            b1 = wp.tile([128, HC], f32, tag="b1")
            b2 = wp.tile([128, HC], f32, tag="b2")
            b3 = wp.tile([128, HC], f32, tag="b3")
            bo = wp.tile([128, 2], f32, tag="bo")
            amat = wp.tile([L, L], f32r, tag="amat")
            bmat = wp.tile([C, L], f32r, tag="bmat")
            ut = wp.tile([C, MSTEPS * BS], f32r, tag="ut")
            # DMA order matters for startup latency: w1 + first input tile
            # first (they gate the first matmuls), big weights split into
            # 4 chunks so they spread across DMA queues instead of
            # serializing ~1MB on one queue each.
            nc.sync.dma_start(out=w1, in_=w1_d)
            for sb_t, dr in ((b1, b1_d), (b2, b2_d), (b3, b3_d), (bo, bo_d)):
                nc.sync.dma_start(out=sb_t, in_=dr)
            ut_loaded = False

            def dma_chunked(sb_t, dr, nchunks):
                w = dr.shape[-1]
                cw = w // nchunks
                for k in range(nchunks):
                    nc.sync.dma_start(out=sb_t[:, k * cw:(k + 1) * cw],
                                      in_=dr[:, k * cw:(k + 1) * cw].bitcast(sb_t.dtype))

            relu_ix = [0]  # round-robin relu between ScalarE and VectorE

            def relu_bias(out_ap, psum_ap, bias_ap):
                if relu_ix[0] % 2 == 0:
                    nc.scalar.activation(out_ap, psum_ap, AT.Relu, bias=bias_ap)
                else:
                    nc.vector.tensor_scalar(
                        out=out_ap, in0=psum_ap, scalar1=bias_ap, scalar2=0.0,
                        op0=ALU.add, op1=ALU.max)
                relu_ix[0] += 1

            def stage1(c0, n):
                """Load input columns [c0, c0+n) and run layer 1 -> h1 chunks.
                Emitted one tile AHEAD of the rest of the encoder so the PE
                has independent matmuls to chew on while layer-1 relus land."""
                xin = xp.tile([128, NT], bf16, tag="xin", name=f"xin_{c0}")
                load_replicated(xin, c0, n)
                h1 = []
                for c in range(HC):
                    ps = hps.tile([128, NT], f32, tag="hps", name=f"ps1_{c0}_{c}")
                    nc.tensor.matmul(ps[:, :n], w1[32 * c:32 * (c + 1), :],
                                     xin[32 * c:32 * (c + 1), :n],
                                     start=True, stop=True,
                                     tile_position=(32 * c, 0))
                    sb = h1p.tile([128, NT], bf16, tag="h1", name=f"h1_{c0}_{c}")
                    relu_bias(sb[:, :n], ps[:, :n], b1[:, c:c + 1])
                    h1.append(sb)
                return h1

            def encoder_rest(h1, n):
                """Layers 2..4 for a tile whose stage1 ran earlier."""
                prev = h1
                for w, b, pool, tag in ((w2, b2, h2p, "h2"), (w3, b3, h3p, "h3")):
                    cur = []
                    for co in range(HC):
                        ps = hps.tile([128, NT], f32, tag="hps", name=f"ps_{tag}_{co}")
                        for ci in range(HC):
                            nc.tensor.matmul(
                                ps[:, :n],
                                w[:, H * ci + 128 * co: H * ci + 128 * (co + 1)],
                                prev[ci][:, :n],
                                start=(ci == 0), stop=(ci == HC - 1))
                        sb = pool.tile([128, NT], bf16, tag=tag, name=f"sb_{tag}_{co}")
                        relu_bias(sb[:, :n], ps[:, :n], b[:, co:co + 1])
                        cur.append(sb)
                    prev = cur
                # output layer -> [128, n] psum; rows E..127 are zero padding
                ps = hps.tile([128, NT], f32, tag="hps", name="eps_t")
                for ci in range(HC):
                    nc.tensor.matmul(ps[:, :n], wo[:, 128 * ci:128 * (ci + 1)],
                                     prev[ci][:, :n],
                                     start=(ci == 0), stop=(ci == HC - 1))
                return ps

            import concourse.bass as bass

            def load_replicated(xin, c0, n):
                """DMA xt_d[:, c0:c0+n] into xin [128, n], replicated 4x on
                the partition axis (one DMA per replica -> parallel queues)."""
                for c in range(HC):
                    nc.sync.dma_start(out=xin[32 * c:32 * (c + 1), :n],
                                      in_=xt_d[:, c0:c0 + n])

            # ---- x_k tile -> z0 ----
            dma_chunked(ut, ut_d, 4)
            h1_xk = stage1(0, BS)
            dma_chunked(w2, w2_d, 4)
            dma_chunked(w3, w3_d, 4)
            dma_chunked(wo, wo_d, 2)
            nc.sync.dma_start(out=amat, in_=a_d.bitcast(f32r))
            nc.sync.dma_start(out=bmat, in_=bm_d.bitcast(f32r))
            h1_cur = stage1(BS, NT)  # tile 0, one ahead
            e_ps = encoder_rest(h1_xk, BS)
            exk = ep.tile([E, NT], f32, tag="exk")
            nc.vector.tensor_scalar_add(exk[:, :BS], e_ps[:E, :BS], bo[:E, 0:1])
            z_prev = zpl.tile([L, BS], f32r, tag="z")
            nc.sync.dma_start(out=z_prev[:S, :], in_=xk32_d.bitcast(f32r))
            # SBUF->SBUF DMA to shift encoder output down 32 partitions
            nc.sync.dma_start(out=z_prev[S:, :], in_=exk[:, :BS].bitcast(f32r))

            # ---- main loop: 32 encoder tiles, 2 scan steps interleaved ----
            # Each psum block serves TWO scan steps: one N=512 Bmat matmul
            # computes bu for steps 2m and 2m+1, then the two serial A
            # matmuls accumulate into the respective halves.
            state = {"m": 0, "z": z_prev, "ps": None}

            def scan_step():
                m = state["m"]
                if m >= MSTEPS:
                    return
                if m % 2 == 0:
                    ps = sps.tile([L, 2 * BS], f32, tag="sps")
                    nc.tensor.matmul(ps, bmat,
                                     ut[:, m * BS:(m + 2) * BS],
                                     start=True, stop=False)
                    state["ps"] = ps
                else:
                    ps = state["ps"]
                half = ps[:, (m % 2) * BS:(m % 2 + 1) * BS]
                nc.tensor.matmul(half, amat, state["z"],
                                 start=False, stop=(m % 2 == 1))
                z_new = zpl.tile([L, BS], f32r, tag="z")
                nc.vector.tensor_copy(z_new, half)
                nc.sync.dma_start(out=zp_d[m].bitcast(f32r), in_=z_new)
                state["z"] = z_new
                state["m"] = m + 1

            for t in range(N_TILES):
                h1_next = (stage1(BS + (t + 1) * NT, NT)
                           if t + 1 < N_TILES else None)
                # scan steps bracket the encoder body so consecutive steps
                # have ~36 matmuls between them (hides the psum->sbuf->PE
                # round-trip of the serial z chain)
                if t >= 1:
                    scan_step()
                e_ps = encoder_rest(h1_cur, NT)
                esb = ep.tile([E, NT], f32, tag="esb")
                nc.vector.tensor_scalar_add(esb, e_ps[:E], bo[:E, 0:1])
                nc.sync.dma_start(out=et_d[:, t * NT:(t + 1) * NT], in_=esb)
                if t >= 1:
                    scan_step()
                h1_cur = h1_next
            while state["m"] < MSTEPS:
                scan_step()

    nc.compile()
    return nc


def _get_program():
    if "nc" not in _CACHE:
        _CACHE["nc"] = _build_program()
    return _CACHE["nc"]


def kernel(x_k, u_seq, x_next_seq, W1, b1, W2, b2, W3, b3, Wo, bo, A, Bmat):
    outs, _ = run_kernel_internal(
        x_k, u_seq, x_next_seq, W1, b1, W2, b2, W3, b3, Wo, bo, A, Bmat)
    return outs


def run_kernel_internal(x_k, u_seq, x_next_seq, W1, b1, W2, b2, W3, b3,
                        Wo, bo, A, Bmat, **spmd_kwargs):
    from concourse import bass_utils

    x_k = np.asarray(x_k, dtype=np.float32)
    u_seq = np.asarray(u_seq, dtype=np.float32)
    x_next_seq = np.asarray(x_next_seq, dtype=np.float32)
    W1 = np.asarray(W1, dtype=np.float32)
    W2 = np.asarray(W2, dtype=np.float32)
    W3 = np.asarray(W3, dtype=np.float32)
    Wo = np.asarray(Wo, dtype=np.float32)
    b1 = np.asarray(b1, dtype=np.float32)
    b2 = np.asarray(b2, dtype=np.float32)
    b3 = np.asarray(b3, dtype=np.float32)
    bo = np.asarray(bo, dtype=np.float32)
    A = np.asarray(A, dtype=np.float32)
    Bmat = np.asarray(Bmat, dtype=np.float32)

    # ---- replicated weight layouts (encoder in bf16) ----
    import ml_dtypes

    bf = ml_dtypes.bfloat16
    w1b = np.ascontiguousarray(
        W1.reshape(S, HC, 128).transpose(1, 0, 2).reshape(128, 128)).astype(bf)
    w2r = np.ascontiguousarray(
        W2.reshape(HC, 128, H).transpose(1, 0, 2).reshape(128, HC * H)).astype(bf)
    w3r = np.ascontiguousarray(
        W3.reshape(HC, 128, H).transpose(1, 0, 2).reshape(128, HC * H)).astype(bf)
    wo_pad = np.zeros((H, 128), dtype=np.float32)
    wo_pad[:, :E] = Wo
    wor = np.ascontiguousarray(
        wo_pad.reshape(HC, 128, 128).transpose(1, 0, 2).reshape(128, HC * 128)
    ).astype(bf)
    b1r = np.ascontiguousarray(b1.reshape(HC, 128).T)
    b2r = np.ascontiguousarray(b2.reshape(HC, 128).T)
    b3r = np.ascontiguousarray(b3.reshape(HC, 128).T)
    bor = np.zeros((128, 2), dtype=np.float32)
    bor[:E, 0] = bo
    bor[S:, 1] = bo

    shared = {"w1": w1b, "w2": w2r, "w3": w3r, "wo": wor,
              "b1": b1r, "b2": b2r, "b3": b3r, "bo": bor,
              "a": A, "bm": Bmat}

    in_maps = []
    for i in range(N_CORES):
        sl = slice(i * BS, (i + 1) * BS)
        xkT = x_k[sl].T                                        # [32, 256]
        xnT = x_next_seq[sl].reshape(RT, S).T                  # [32, 16384]
        xt = np.ascontiguousarray(
            np.concatenate([xkT, xnT], axis=1)).astype(bf)
        xk32 = np.ascontiguousarray(xkT)
        ut = np.ascontiguousarray(
            u_seq[sl].transpose(2, 1, 0).reshape(C, MSTEPS * BS))
        in_maps.append({"xt": xt, "xk32": xk32, "ut": ut, **shared})

    nc = _get_program()
    res = bass_utils.run_bass_kernel_spmd(
        nc, in_maps, core_ids=list(range(N_CORES)), **spmd_kwargs)

    z_pred = np.empty((B, MSTEPS, L), dtype=np.float32)
    z_target = np.empty((B, MSTEPS, L), dtype=np.float32)
    for i in range(N_CORES):
        sl = slice(i * BS, (i + 1) * BS)
        out = res.results[i]
        z_pred[sl] = out["zp"].transpose(2, 0, 1)              # [256, 64, 128]
        z_target[sl, :, :S] = x_next_seq[sl]
        z_target[sl, :, S:] = out["et"].T.reshape(BS, MSTEPS, E)
    x_pred = np.ascontiguousarray(z_pred[..., :S])
    return (z_pred, x_pred, z_target), res
